# revision 30
# baseline (speedup 1.0000x reference)
"""MultiHeadAttention (qk-LayerNorm + RoPE) Trainium2 kernel, 8 NeuronCores.

Sharding: batch (4) x head-group (2x8 heads). Core c handles batch c//2,
heads 8*(c%2) .. 8*(c%2)+7. Each core computes QKV projections for its
batch restricted to its head group, per-head LayerNorm + rotary embedding,
attention, and a partial output projection over its 512 context channels.
The two partial o_proj results per batch are summed on the host (the
"unshard" step), which keeps the device program collective-free.

Dataflow per core (all matmul operands bf16, PSUM accumulation fp32):
  Startup: ACT spline table sets (exp, sqrt) preloaded inside the
    input-DMA shadow; rope tables right after Wq so tile 0's LN/rope
    chain unblocks early.
  Phase 1 (per 128-token tile, emitted per tile so it can interleave):
    QKV projections through a single 2-bank rotating PSUM slot, q/k
    copied to a bf16 SBUF shadow (ACT), LN sum/sumsq reduced on DVE with
    the squares on gpsimd, LN-apply + rope in place, q/k transposed to
    [d, t] via PE transposes whose PSUM targets share the QKV slot pool.
    Tiles 0-3 are emitted up front; tiles 4-7 are WOVEN into the first
    attention block's slots, so their QKV/LN work fills the PE/ACT gaps
    of the exp stream instead of serializing in front of it.
  Phase 2 (m-blocked attention: 64 slots = m-half x pair x j-tile):
    scores for one m-half into alternating 2-bank PSUM tiles (heads
    concurrent in distinct PE row groups), one N=1024 exp ACT op per
    slot streaming back-to-back; ctx matmuls (ones column in v yields
    the softmax denominator) trail by 3 slots on a deque that carries
    across block boundaries so the PE never idles into a HAM
    re-throttle. ctx accumulators are single-bank [65, 512] per head.
    Normalize per (pair, m): denominator copy, reciprocal_approx_fast,
    gpsimd partition_broadcast, one multiply per head; the odd head
    reaches partitions 64:127 via an SBUF-to-SBUF DMA hop. Warm-keeper
    matmuls bridge the final normalize gap.
  Phase 3: o_proj per token tile (4x2 accumulating K=128 matmuls), DVE
    psum drain, bf16 partial DMA'd to DRAM (host sums pair partials).
"""
import sys

for _p in ("/opt/trn_rl_repo", "/root/.axon_site", "/root/.axon_site/_ro/trn_rl_repo",
           "/root/.axon_site/_ro/pypackages"):
    if _p not in sys.path:
        sys.path.append(_p)

import numpy as np

import concourse.bass as bass
import concourse.tile as tile
from concourse import bacc, mybir
from concourse.bass_utils import run_bass_kernel_spmd
from concourse.masks import make_identity

F32 = mybir.dt.float32
BF16 = mybir.dt.bfloat16
P = 128
B, L, C, H, D = 4, 1024, 1024, 16, 64
HC = 8          # heads per core
NPR = HC // 2   # head pairs per core
CG = HC * D     # 512 context channels per core
NT = L // P     # 8 token tiles
NCK = C // P    # 8 contraction tiles
THETA = 50000.0
EPS = 1e-5

_NC_CACHE = {}


def _build_nc():
    nc = bacc.Bacc("TRN2", target_bir_lowering=False, debug=False, num_devices=8)

    xT_d = nc.dram_tensor("xT", [P, NT, NCK, P], BF16, kind="ExternalInput")
    wqT_d = nc.dram_tensor("wqT", [C, CG], BF16, kind="ExternalInput")
    wkT_d = nc.dram_tensor("wkT", [C, CG], BF16, kind="ExternalInput")
    wvT_d = nc.dram_tensor("wvT", [C, CG], BF16, kind="ExternalInput")
    woT_d = nc.dram_tensor("woT", [NPR, P, C], BF16, kind="ExternalInput")
    aq_d = nc.dram_tensor("aq", [P, NT, D], BF16, kind="ExternalInput")
    bq_d = nc.dram_tensor("bq", [P, NT, D], BF16, kind="ExternalInput")
    ak_d = nc.dram_tensor("ak", [P, NT, D], BF16, kind="ExternalInput")
    bk_d = nc.dram_tensor("bk", [P, NT, D], BF16, kind="ExternalInput")
    out_d = nc.dram_tensor("out", [L, C], BF16, kind="ExternalOutput")

    with tile.TileContext(nc) as tc:
        with (
            tc.tile_pool(name="const", bufs=1) as constp,
            tc.tile_pool(name="w", bufs=1) as wpool,
            tc.tile_pool(name="big", bufs=1) as bigp,
            tc.tile_pool(name="scr", bufs=2) as scrp,
            tc.tile_pool(name="rope", bufs=2) as ropep,
            tc.tile_pool(name="stat", bufs=2) as statp,
            tc.tile_pool(name="exp", bufs=1) as expp,
            tc.tile_pool(name="den", bufs=2) as denp,
            tc.tile_pool(name="fin", bufs=2) as finp,
        ):
            ident = constp.tile([P, P], BF16)
            make_identity(nc, ident)
            eps_t = constp.tile([P, 1], F32)
            nc.vector.memset(eps_t[:], EPS)
            # preload the ACT spline table sets (exp, sqrt) inside the
            # startup DMA shadow
            tblw = constp.tile([P, 1], F32)
            nc.scalar.activation(tblw[:], eps_t[:],
                                 mybir.ActivationFunctionType.Exp)
            nc.scalar.activation(tblw[:], eps_t[:],
                                 mybir.ActivationFunctionType.Sqrt,
                                 bias=eps_t[:])

            a2_t = constp.tile([P, 2, NT, D], BF16)
            b2_t = constp.tile([P, 2, NT, D], BF16)

            xt_all = bigp.tile([P, NT, NCK, P], BF16)
            wq_t, wk_t, wv_t = [], [], []

            def _w_dmas(lst, nm, d_):
                for ck in range(NCK):
                    t_ = wpool.tile([P, CG], BF16, tag=f"{nm}{ck}", name=f"{nm}{ck}")
                    nc.sync.dma_start(
                        t_[:],
                        d_.ap().rearrange("(k p) o -> p k o", p=P)[:, ck, :])
                    lst.append(t_)

            nc.sync.dma_start(xt_all[:, 0], xT_d.ap()[:, 0])
            _w_dmas(wq_t, "wq", wqT_d)
            nc.sync.dma_start(a2_t[:, 0, :, :], aq_d.ap())
            nc.sync.dma_start(b2_t[:, 0, :, :], bq_d.ap())
            _w_dmas(wk_t, "wk", wkT_d)
            nc.sync.dma_start(a2_t[:, 1, :, :], ak_d.ap())
            nc.sync.dma_start(b2_t[:, 1, :, :], bk_d.ap())
            nc.sync.dma_start(xt_all[:, 1], xT_d.ap()[:, 1])
            _w_dmas(wv_t, "wv", wvT_d)
            for ti in range(2, NT):
                nc.sync.dma_start(xt_all[:, ti], xT_d.ap()[:, ti])

            # v with a ones column appended per head: [s_tile, j, head, 65]
            v_sb = bigp.tile([P, NT, HC, D + 1], BF16)
            nc.vector.memset(
                v_sb[:, :, :, D:D + 1].rearrange("p t h o -> p (t h) o"), 1.0)

            qT_pack = bigp.tile([P, NPR, L], BF16)
            kT_pack = bigp.tile([P, NPR, L], BF16)
            ctxT2 = bigp.tile([P, NPR, L], BF16)

            h_ = D // 2
            norm_tmp = {}
            wo_l = []

            with tc.tile_pool(name="qkv", bufs=2, space="PSUM") as qkvp, \
                 tc.tile_pool(name="pss", bufs=1, space="PSUM") as pssp, \
                 tc.tile_pool(name="psc", bufs=1, space="PSUM") as pscp:

                # ---------- Phase 1 unit: one token tile ----------
                def emit_p1(ti):
                    qk_sb = scrp.tile([P, 2, HC, D], BF16, tag="qk_sb",
                                      name="qk_sb")
                    stats = statp.tile([P, 4, HC], F32, tag="stats",
                                       name="stats")
                    for i, w_t in enumerate((wq_t, wk_t)):
                        ps_ = qkvp.tile([P, CG], F32, tag="qkv", name="psqk")
                        for ck in range(NCK):
                            nc.tensor.matmul(
                                ps_[:], xt_all[:, ti, ck, :], w_t[ck][:],
                                start=(ck == 0), stop=(ck == NCK - 1))
                        nc.scalar.copy(qk_sb[:, i], ps_[:].rearrange(
                            "p (h d) -> p h d", d=D))
                        nc.vector.reduce_sum(
                            stats[:, i, :], qk_sb[:, i],
                            axis=mybir.AxisListType.X)
                        sq = scrp.tile([P, HC, D], BF16, tag="sq", name="sq")
                        nc.gpsimd.tensor_mul(sq[:], qk_sb[:, i], qk_sb[:, i])
                        nc.vector.reduce_sum(
                            stats[:, 2 + i, :], sq[:],
                            axis=mybir.AxisListType.X)
                    psv = qkvp.tile([P, CG], F32, tag="qkv", name="psv")
                    for ck in range(NCK):
                        nc.tensor.matmul(
                            psv[:], xt_all[:, ti, ck, :], wv_t[ck][:],
                            start=(ck == 0), stop=(ck == NCK - 1))
                    nc.scalar.copy(
                        v_sb[:, ti, :, 0:D],
                        psv[:].rearrange("p (h d) -> p h d", d=D))

                    mu2 = statp.tile([P, 4, HC], F32, tag="mu2", name="mu2")
                    nc.vector.tensor_scalar_mul(mu2[:], stats[:], 1.0 / D)
                    var = statp.tile([P, 2, HC], F32, tag="var", name="var")
                    nc.vector.tensor_mul(var[:], mu2[:, 0:2, :], mu2[:, 0:2, :])
                    nc.vector.tensor_sub(var[:], mu2[:, 2:4, :], var[:])
                    std = statp.tile([P, 2, HC], F32, tag="std", name="std")
                    nc.scalar.activation(std[:], var[:],
                                         mybir.ActivationFunctionType.Sqrt,
                                         bias=eps_t[:])
                    inv = statp.tile([P, 2, HC], F32, tag="inv", name="inv")
                    nc.vector.reciprocal(inv[:], std[:])
                    invh = statp.tile([P, 2, HC], BF16, tag="invh", name="invh")
                    nc.vector.tensor_copy(invh[:], inv[:])
                    shifth = statp.tile([P, 2, HC], BF16, tag="shifth",
                                        name="shifth")
                    nc.vector.tensor_mul(shifth[:], mu2[:, 0:2, :], inv[:])

                    inv_b = invh[:].rearrange("p i h -> p i h ()").to_broadcast(
                        (P, 2, HC, D))
                    sh_b = shifth[:].rearrange("p i h -> p i h ()").to_broadcast(
                        (P, 2, HC, D))
                    a_b = a2_t[:, :, ti, :].rearrange(
                        "p i d -> p i () d").to_broadcast((P, 2, HC, D))
                    t1 = qk_sb
                    nc.vector.tensor_mul(t1[:], t1[:], inv_b)
                    nc.vector.tensor_sub(t1[:], t1[:], sh_b)
                    rope = ropep.tile([P, 2, HC, D], BF16, tag="rope",
                                      name="rope")
                    nc.vector.tensor_mul(rope[:], t1[:], a_b)
                    r2 = scrp.tile([P, 2, HC, D], BF16, tag="r2", name="r2")
                    nc.vector.tensor_mul(
                        r2[:, :, :, 0:h_], t1[:, :, :, h_:D],
                        b2_t[:, :, ti, 0:h_].rearrange(
                            "p i d -> p i () d").to_broadcast((P, 2, HC, h_)))
                    nc.vector.tensor_mul(
                        r2[:, :, :, h_:D], t1[:, :, :, 0:h_],
                        b2_t[:, :, ti, h_:D].rearrange(
                            "p i d -> p i () d").to_broadcast((P, 2, HC, h_)))
                    nc.vector.tensor_add(rope[:], rope[:], r2[:])
                    for i, dstpack in ((0, qT_pack), (1, kT_pack)):
                        ps_t = qkvp.tile([P, NPR, P], BF16, tag="qkv",
                                         name="ps_t")
                        for pr2 in range(NPR):
                            nc.tensor.transpose(
                                ps_t[:, pr2, :],
                                rope[:, i, 2 * pr2:2 * pr2 + 2, :].rearrange(
                                    "p h d -> p (h d)"),
                                ident[:])
                        nc.scalar.copy(dstpack[:, :, bass.ts(ti, P)], ps_t[:])

                for ti in range(4):
                    emit_p1(ti)

                # ---------- Phase 2: m-blocked attention stream ----------
                scale = float(D) ** -0.5
                pss_ab = [pssp.tile([P, 2, 512], F32, name=f"pss{a}")
                          for a in range(2)]
                pscs_of = {}
                exps_of = {}
                pending = []

                def normalize2(m, pr, pscs):
                    rbs = []
                    for head in range(2):
                        den = denp.tile([1, 512], F32, tag=f"den{head}",
                                        name=f"den{head}")
                        nc.vector.tensor_copy(den[:], pscs[head][D:D + 1, :])
                        dr = denp.tile([1, 512], F32, tag=f"dr{head}",
                                       name=f"dr{head}")
                        nc.vector.reciprocal_approx_fast(dr[:], den[:])
                        rb = denp.tile([D, 512], F32, tag=f"rb{head}",
                                       name=f"rb{head}")
                        nc.gpsimd.partition_broadcast(rb[:], dr[:])
                        rbs.append(rb)
                    nc.vector.tensor_mul(
                        ctxT2[0:D, pr, bass.ts(m, 512)],
                        pscs[0][0:D, :], rbs[0][:])
                    tmpB = denp.tile([D, 512], BF16, tag="tmpB", name="tmpB")
                    nc.vector.tensor_mul(tmpB[:], pscs[1][0:D, :], rbs[1][:])
                    nc.sync.dma_start(
                        ctxT2[D:2 * D, pr, bass.ts(m, 512)], tmpB[:])
                    norm_tmp["tmpB"] = tmpB

                def ctx_mm(m, cpr, jc):
                    for head in range(2):
                        h = 2 * cpr + head
                        nc.tensor.matmul(
                            pscs_of[(m, cpr)][head][:],
                            v_sb[:, jc, h, :],
                            exps_of[(m, cpr)][jc][:, head, :],
                            start=(jc == 0), stop=(jc == NT - 1))
                    if jc == NT - 1:
                        normalize2(m, cpr, pscs_of[(m, cpr)])

                slot = 0
                for m in range(2):
                    for pr in range(NPR):
                        pscs_of[(m, pr)] = [
                            pscp.tile([D + 1, 512], F32, tag=f"pc{h}",
                                      name=f"pc{h}") for h in range(2)]
                        exps_of[(m, pr)] = []
                        for j in range(NT):
                            pss = pss_ab[slot % 2]
                            slot += 1
                            for half in range(2):
                                nc.tensor.matmul(
                                    pss[:, half, :],
                                    kT_pack[half * D:(half + 1) * D, pr,
                                            bass.ts(j, P)],
                                    qT_pack[half * D:(half + 1) * D, pr,
                                            bass.ts(m, 512)],
                                    start=True, stop=True)
                            expj = expp.tile([P, 2, 512], BF16, tag=f"exp{j}",
                                             name=f"exp{j}")
                            nc.scalar.activation(
                                expj[:], pss[:],
                                mybir.ActivationFunctionType.Exp, scale=scale)
                            exps_of[(m, pr)].append(expj)
                            pending.append((m, pr, j))
                            if len(pending) > 3:
                                ctx_mm(*pending.pop(0))
                            # weave phase-1 tiles 4-7 into the first block:
                            # their QKV/LN chains fill the exp stream's PE
                            # and ACT gaps instead of serializing before it.
                            # tile 4+u is emitted at slot 2u, one full slot
                            # before the first scores matmul that reads its
                            # kT (slot 4+u) sits in the program order
                            if m == 0 and pr == 0 and j in (0, 2, 4, 6):
                                emit_p1(4 + j // 2)
                        if m == 0 and pr == NPR - 1:
                            # o_proj weights mid-stream: reuses the wq slots
                            # (dead once tile 7's q projection is done)
                            for pr2 in range(NPR):
                                wo_p = wpool.tile([P, C], BF16, tag=f"wq{pr2}",
                                                  name=f"wo{pr2}")
                                nc.sync.dma_start(wo_p[:], woT_d.ap()[pr2, :, :])
                                wo_l.append(wo_p)
                while pending:
                    ctx_mm(*pending.pop(0))

                # keep the PE warm across the final normalize chain
                xt_row = xt_all[:, 0].rearrange("p a b -> p (a b)")
                for _ in range(3):
                    nc.tensor.matmul(pss_ab[0][:, 0, :],
                                     xt_all[:, 0, 0, :], xt_row[:, 0:512],
                                     start=True, stop=True)
                tmpB_last = norm_tmp["tmpB"]
                for _ in range(3):
                    nc.tensor.matmul(
                        pss_ab[0][:, 1, :],
                        tmpB_last[:, 0:P], tmpB_last[:, :],
                        start=True, stop=True)

            # ---------------- Phase 3: output projection --------------------
            with tc.tile_pool(name="pso", bufs=2, space="PSUM") as psop:
                for ti in range(NT):
                    pso = psop.tile([P, C], F32, name="pso")
                    for pr in range(NPR):
                        for m in range(2):
                            nc.tensor.matmul(
                                pso[:, bass.ts(m, 512)],
                                ctxT2[:, pr, bass.ts(ti, P)],
                                wo_l[pr][:, bass.ts(m, 512)],
                                start=(pr == 0), stop=(pr == NPR - 1))
                    out_sb = finp.tile([P, C], BF16, tag="out", name="out_sb")
                    nc.vector.tensor_copy(out_sb[:], pso[:])
                    nc.sync.dma_start(out_d.ap()[bass.ts(ti, P), :], out_sb[:])

    nc.compile()
    return nc


def _rope_tables(w, b):
    """A[t,d], B[t,d] with the rotate-half sign folded into B."""
    inv_freq = 1.0 / THETA ** (np.arange(0, D, 2, dtype=np.float64) / D)
    freqs = np.arange(L, dtype=np.float64)[:, None] * inv_freq[None, :]
    freqs = np.concatenate([freqs, freqs], axis=1)           # [L, D]
    cos, sin = np.cos(freqs), np.sin(freqs)
    w = w.astype(np.float64)
    w_rot = np.concatenate([w[D // 2:], w[:D // 2]])
    sgn = np.concatenate([-np.ones(D // 2), np.ones(D // 2)])
    A = (cos * w[None, :]).astype(np.float32)
    Bt = (sin * w_rot[None, :] * sgn[None, :]).astype(np.float32)
    if np.any(b != 0):
        raise NotImplementedError("nonzero qk-norm bias not supported")
    return A, Bt


def _make_in_maps(inputs):
    from ml_dtypes import bfloat16

    x = np.asarray(inputs["q"], dtype=np.float32)
    Wq = np.asarray(inputs["Wq"], dtype=np.float32)
    Wk = np.asarray(inputs["Wk"], dtype=np.float32)
    Wv = np.asarray(inputs["Wv"], dtype=np.float32)
    Wo = np.asarray(inputs["Wo"], dtype=np.float32)
    bo = np.asarray(inputs["bo"], dtype=np.float32)
    assert not np.any(bo != 0), "nonzero output bias not supported"

    Aq, Bq = _rope_tables(np.asarray(inputs["qn_w"], np.float32),
                          np.asarray(inputs["qn_b"], np.float32))
    Ak, Bk = _rope_tables(np.asarray(inputs["kn_w"], np.float32),
                          np.asarray(inputs["kn_b"], np.float32))
    WoT = np.ascontiguousarray(Wo.T)                          # [C(c'), C(o)]

    def _tbl(a):   # [L, D] -> [P, NT, D] (partition-major, contiguous DMA)
        return np.ascontiguousarray(
            a.reshape(NT, P, D).transpose(1, 0, 2)).astype(bfloat16)
    Aqr, Bqr, Akr, Bkr = _tbl(Aq), _tbl(Bq), _tbl(Ak), _tbl(Bk)

    in_maps = []
    for c in range(8):
        b_, g = c // 2, c % 2
        sl = slice(g * CG, (g + 1) * CG)
        in_maps.append({
            "xT": np.ascontiguousarray(
                x[b_].T.reshape(NCK, P, NT, P).transpose(1, 2, 0, 3)).astype(bfloat16),
            "wqT": np.ascontiguousarray(Wq[sl, :].T).astype(bfloat16),
            "wkT": np.ascontiguousarray(Wk[sl, :].T).astype(bfloat16),
            "wvT": np.ascontiguousarray(Wv[sl, :].T).astype(bfloat16),
            # [pair, 2*D rows (= the pair's context channels), C]
            "woT": np.ascontiguousarray(
                WoT[sl, :].reshape(NPR, P, C)).astype(bfloat16),
            "aq": Aqr, "bq": Bqr, "ak": Akr, "bk": Bkr,
        })
    return in_maps


def kernel(**inputs):
    in_maps = _make_in_maps(inputs)

    if "nc" not in _NC_CACHE:
        _NC_CACHE["nc"] = _build_nc()
    nc = _NC_CACHE["nc"]

    B = 4
    res = run_bass_kernel_spmd(nc, in_maps, core_ids=list(range(8)))
    # each core wrote its full [L, C] o_proj partial; unshard = sum the two
    # head-group partials per batch
    out = np.empty((B, L, C), dtype=np.float32)
    for b_ in range(B):
        out[b_] = (res.results[2 * b_]["out"].astype(np.float32)
                   + res.results[2 * b_ + 1]["out"].astype(np.float32))
    return out


# revision 31
# speedup vs baseline: 1.2306x; 1.2306x over previous
"""MultiHeadAttention (qk-LayerNorm + RoPE) Trainium2 kernel, 8 NeuronCores.

Sharding: batch (4) x head-group (2x8 heads). Core c handles batch c//2,
heads 8*(c%2) .. 8*(c%2)+7. Each core computes QKV projections for its
batch restricted to its head group, per-head LayerNorm + rotary embedding,
attention, and a partial output projection over its 512 context channels.
The two partial o_proj results per batch are summed on the host (the
"unshard" step), which keeps the device program collective-free: no NEFF
entry barrier, no ReduceScatter tail.

Dataflow per core (all matmul operands bf16, PSUM accumulation fp32):
  Startup: ACT spline table sets (exp, sqrt) preloaded with dummy
    activations inside the input-DMA shadow; x is loaded tile-major with
    the rope tables right after Wq so tile 0's LN/rope chain unblocks
    early.
  Phase 1: per 128-token tile: QKV projections (bf16), q/k copied to a
    bf16 SBUF shadow by the scalar engine (frees PSUM immediately; LN
    sum/sumsq reduce from the shadow on DVE, the squares on the
    otherwise-idle gpsimd engine), LN-apply + rope in place on the
    shadow, q/k transposed to [d, t] layout via PE transposes.
  Phase 2: per head pair: scores into TWO single-m PSUM tiles (heads
    concurrent in distinct PE row groups); exp as two N=1024 ACT ops
    (one per m tile) so the m0 tile refills with j+1's scores while the
    ACT engine exps the m1 tile - the exp stream runs back-to-back at
    ~1.1us/op. ctx matmuls (ones column in v yields the softmax
    denominator row) trail the stream by 3 slots on a deque that CARRIES
    ACROSS pair boundaries, so pair p's ctx tail executes inside pair
    p+1's first slots and the PE never idles into a HAM re-throttle.
    Normalize per pair:
    denominator row copy, reciprocal_approx_fast on [1, 1024], gpsimd
    partition_broadcast, one multiply per head; the odd head reaches
    partitions 64:127 via an SBUF-to-SBUF DMA hop on the idle sync
    queue. Warm-keeper matmuls (3 at stream drain + 3 chained on the
    last normalize's tmpB) bridge the normalize gap so o_proj starts at
    the full 2.4 GHz clock.
  Phase 3: o_proj per token tile (4x2 accumulating K=128 matmuls), psum
    drained alternately by DVE and ACT, bf16 partial DMA'd to DRAM (the
    host sums the pair partials in fp32).
"""
import sys

for _p in ("/opt/trn_rl_repo", "/root/.axon_site", "/root/.axon_site/_ro/trn_rl_repo",
           "/root/.axon_site/_ro/pypackages"):
    if _p not in sys.path:
        sys.path.append(_p)

import numpy as np

import concourse.bass as bass
import concourse.tile as tile
from concourse import bacc, mybir
from concourse.bass_utils import run_bass_kernel_spmd
from concourse.masks import make_identity

F32 = mybir.dt.float32
F32R = mybir.dt.float32r
BF16 = mybir.dt.bfloat16
P = 128
B, L, C, H, D = 4, 1024, 1024, 16, 64
HC = 8          # heads per core
NPR = HC // 2   # head pairs per core
CG = HC * D     # 512 context channels per core
NT = L // P     # 8 token tiles
NCK = C // P    # 8 contraction tiles
THETA = 50000.0
EPS = 1e-5

_NC_CACHE = {}
# dummy keep-warm matmul counts (fill PE idle so the HAM clock gate stays
# at K=8/8; targets are PSUM slivers cleared by the next start=True group)
WARM1, WARM2, WARM2E, WARM3, WARM3PRE = 0, 0, 0, 0, 0


def _build_nc():
    nc = bacc.Bacc("TRN2", target_bir_lowering=False, debug=False, num_devices=8)

    xT_d = nc.dram_tensor("xT", [P, NT, NCK, P], BF16, kind="ExternalInput")
    wqT_d = nc.dram_tensor("wqT", [C, CG], BF16, kind="ExternalInput")
    wkT_d = nc.dram_tensor("wkT", [C, CG], BF16, kind="ExternalInput")
    wvT_d = nc.dram_tensor("wvT", [C, CG], BF16, kind="ExternalInput")
    woT_d = nc.dram_tensor("woT", [NPR, P, C], BF16, kind="ExternalInput")
    aq_d = nc.dram_tensor("aq", [P, NT, D], BF16, kind="ExternalInput")
    bq_d = nc.dram_tensor("bq", [P, NT, D], BF16, kind="ExternalInput")
    ak_d = nc.dram_tensor("ak", [P, NT, D], BF16, kind="ExternalInput")
    bk_d = nc.dram_tensor("bk", [P, NT, D], BF16, kind="ExternalInput")
    out_d = nc.dram_tensor("out", [L, C], BF16, kind="ExternalOutput")

    with tile.TileContext(nc) as tc:
        with (
            tc.tile_pool(name="const", bufs=1) as constp,
            tc.tile_pool(name="w", bufs=1) as wpool,
            tc.tile_pool(name="big", bufs=1) as bigp,
            tc.tile_pool(name="scr", bufs=2) as scrp,
            tc.tile_pool(name="rope", bufs=2) as ropep,
            tc.tile_pool(name="stat", bufs=2) as statp,
            tc.tile_pool(name="exp", bufs=1) as expp,
            tc.tile_pool(name="den", bufs=2) as denp,
            tc.tile_pool(name="fin", bufs=2) as finp,
        ):
            ident = constp.tile([P, P], BF16)
            make_identity(nc, ident)
            eps_t = constp.tile([P, 1], F32)
            nc.vector.memset(eps_t[:], EPS)
            # preload the ACT spline table sets (exp, sqrt) with dummy
            # activations so the ~1.3us-per-set lazy loads happen inside the
            # startup DMA shadow instead of on the phase-1/phase-2 critical
            # path
            tblw = constp.tile([P, 1], F32)
            nc.scalar.activation(tblw[:], eps_t[:],
                                 mybir.ActivationFunctionType.Exp)
            nc.scalar.activation(tblw[:], eps_t[:],
                                 mybir.ActivationFunctionType.Sqrt,
                                 bias=eps_t[:])

            a2_t = constp.tile([P, 2, NT, D], BF16)
            b2_t = constp.tile([P, 2, NT, D], BF16)

            # x resident in SBUF, tile-major. DMA order: x tile 0, all wq,
            # all wk, x tile 1, all wv, x tiles 2..7 — so tile 0's q stats
            # (the head of the DVE pipeline) are ready after ~1.3MB of
            # traffic instead of the full 5MB
            xt_all = bigp.tile([P, NT, NCK, P], BF16)
            wq_t, wk_t, wv_t = [], [], []

            def _w_dmas(lst, nm, d_):
                for ck in range(NCK):
                    t_ = wpool.tile([P, CG], BF16, tag=f"{nm}{ck}", name=f"{nm}{ck}")
                    nc.sync.dma_start(
                        t_[:],
                        d_.ap().rearrange("(k p) o -> p k o", p=P)[:, ck, :])
                    lst.append(t_)

            nc.sync.dma_start(xt_all[:, 0], xT_d.ap()[:, 0])
            _w_dmas(wq_t, "wq", wqT_d)
            # rope tables early: the ti-0 LN/rope chain needs them ~6us in
            nc.sync.dma_start(a2_t[:, 0, :, :], aq_d.ap())
            nc.sync.dma_start(b2_t[:, 0, :, :], bq_d.ap())
            _w_dmas(wk_t, "wk", wkT_d)
            nc.sync.dma_start(a2_t[:, 1, :, :], ak_d.ap())
            nc.sync.dma_start(b2_t[:, 1, :, :], bk_d.ap())
            nc.sync.dma_start(xt_all[:, 1], xT_d.ap()[:, 1])
            _w_dmas(wv_t, "wv", wvT_d)
            for ti in range(2, NT):
                nc.sync.dma_start(xt_all[:, ti], xT_d.ap()[:, ti])

            # v with a ones column appended per head: [s_tile, j, head, 65]
            v_sb = bigp.tile([P, NT, HC, D + 1], BF16)
            nc.vector.memset(
                v_sb[:, :, :, D:D + 1].rearrange("p t h o -> p (t h) o"), 1.0)
            def warm(n, target):
                for _ in range(n):
                    nc.tensor.matmul(target[0:16, 0:16], xt_all[:, 0, 0, 16:32],
                                     xt_all[:, 0, 0, 0:16], start=True, stop=True)


            qT_pack = bigp.tile([P, NPR, L], BF16)
            kT_pack = bigp.tile([P, NPR, L], BF16)
            # ctx packed two heads per 128 partitions: [128, pair, L]
            ctxT2 = bigp.tile([P, NPR, L], BF16)

            # ---------------- Phase 1: QKV + LN + RoPE + transpose ----------
            # processed two token tiles per group: the QKV matmuls and the
            # PSUM-reading ops (reduces, squares, t1) run per tile, the rest
            # of the LN/rope arithmetic runs as merged [P, 2, 2, HC, *] DVE
            # ops to amortize per-op overhead and pipeline drains
            with tc.tile_pool(name="ps1", bufs=2, space="PSUM") as ps1, \
                 tc.tile_pool(name="pst", bufs=2, space="PSUM") as pst:
                for g in range(NT // 2):
                    stats = statp.tile([P, 2, 4, HC], F32, tag="stats")
                    qk_sb = scrp.tile([P, 2, 2, HC, D], BF16, tag="qk_sb")
                    t1 = qk_sb  # LN-apply runs in place on the bf16 copy
                    psqks = []
                    for s in range(2):
                        ti = 2 * g + s
                        psq = ps1.tile([P, CG], F32, tag="psq", name="psq")
                        psk = ps1.tile([P, CG], F32, tag="psk", name="psk")
                        psv = ps1.tile([P, CG], F32, tag="psv", name="psv")
                        psqks.append((psq, psk))
                        for ps_, w_ in ((psq, wq_t), (psk, wk_t), (psv, wv_t)):
                            for ck in range(NCK):
                                nc.tensor.matmul(
                                    ps_[:], xt_all[:, ti, ck, :], w_[ck][:],
                                    start=(ck == 0), stop=(ck == NCK - 1))

                        # v straight to SBUF (bf16); ACT to keep DVE free
                        nc.scalar.copy(
                            v_sb[:, ti, :, 0:D],
                            psv[:].rearrange("p (h d) -> p h d", d=D))

                        # q/k copied to bf16 SBUF (ACT); PSUM frees right
                        # after the copy and the stats reductions run on the
                        # copy, per tensor so the q-side chain starts as soon
                        # as the q projection lands (k weights arrive later).
                        # squares on the otherwise-idle gpsimd engine.
                        sq = scrp.tile([P, 2, HC, D], BF16, tag="sq")
                        for i, ps_ in enumerate((psq, psk)):
                            nc.scalar.copy(qk_sb[:, s, i], ps_[:].rearrange(
                                "p (h d) -> p h d", d=D))
                            nc.vector.reduce_sum(
                                stats[:, s, i, :], qk_sb[:, s, i],
                                axis=mybir.AxisListType.X)
                            nc.gpsimd.tensor_mul(sq[:, i], qk_sb[:, s, i],
                                                 qk_sb[:, s, i])
                            nc.vector.reduce_sum(
                                stats[:, s, 2 + i, :], sq[:, i],
                                axis=mybir.AxisListType.X)
                    mu2 = statp.tile([P, 2, 4, HC], F32, tag="mu2")
                    nc.vector.tensor_scalar_mul(mu2[:], stats[:], 1.0 / D)
                    var = statp.tile([P, 2, 2, HC], F32, tag="var")
                    nc.vector.tensor_mul(var[:], mu2[:, :, 0:2, :], mu2[:, :, 0:2, :])
                    nc.vector.tensor_sub(var[:], mu2[:, :, 2:4, :], var[:])
                    std = statp.tile([P, 2, 2, HC], F32, tag="std")
                    nc.scalar.activation(std[:], var[:],
                                         mybir.ActivationFunctionType.Sqrt,
                                         bias=eps_t[:])
                    inv = statp.tile([P, 2, 2, HC], F32, tag="inv")
                    nc.vector.reciprocal(inv[:], std[:])
                    invh = statp.tile([P, 2, 2, HC], BF16, tag="invh")
                    nc.vector.tensor_copy(invh[:], inv[:])
                    shifth = statp.tile([P, 2, 2, HC], BF16, tag="shifth")
                    nc.vector.tensor_mul(shifth[:], mu2[:, :, 0:2, :], inv[:])

                    h_ = D // 2
                    for s in range(2):
                        ti = 2 * g + s
                        inv_b = invh[:, s].rearrange("p i h -> p i h ()").to_broadcast(
                            (P, 2, HC, D))
                        sh_b = shifth[:, s].rearrange("p i h -> p i h ()").to_broadcast(
                            (P, 2, HC, D))
                        a_b = a2_t[:, :, ti, :].rearrange(
                            "p i d -> p i () d").to_broadcast((P, 2, HC, D))
                        nc.vector.tensor_mul(t1[:, s], t1[:, s], inv_b)
                        nc.vector.tensor_sub(t1[:, s], t1[:, s], sh_b)
                        rope = ropep.tile([P, 2, HC, D], BF16, tag=f"rope{s}")
                        nc.vector.tensor_mul(rope[:], t1[:, s], a_b)
                        r2 = scrp.tile([P, 2, HC, D], BF16, tag=f"r2{s}")
                        nc.vector.tensor_mul(
                            r2[:, :, :, 0:h_], t1[:, s, :, :, h_:D],
                            b2_t[:, :, ti, 0:h_].rearrange(
                                "p i d -> p i () d").to_broadcast((P, 2, HC, h_)))
                        nc.vector.tensor_mul(
                            r2[:, :, :, h_:D], t1[:, s, :, :, 0:h_],
                            b2_t[:, :, ti, h_:D].rearrange(
                                "p i d -> p i () d").to_broadcast((P, 2, HC, h_)))
                        nc.vector.tensor_add(rope[:], rope[:], r2[:])
                        for i, dstpack in ((0, qT_pack), (1, kT_pack)):
                            ps_t = pst.tile([P, NPR, P], BF16, name="ps_t")
                            for pr in range(NPR):
                                nc.tensor.transpose(
                                    ps_t[:, pr, :],
                                    rope[:, i, 2 * pr:2 * pr + 2, :].rearrange(
                                        "p h d -> p (h d)"),
                                    ident[:])
                            nc.scalar.copy(
                                dstpack[:, :, bass.ts(ti, P)], ps_t[:])
                    warm(WARM1, psqks[0][0])

            # o_proj weights early: reuses the per-ck wq slots (dead after
            # phase 1); packed per head pair [128, C] to match ctxT2
            wo_l = []
            for pr in range(NPR):
                wo_p = wpool.tile([P, C], BF16, tag=f"wq{pr}", name=f"wo{pr}")
                nc.sync.dma_start(wo_p[:], woT_d.ap()[pr, :, :])
                wo_l.append(wo_p)

            # ---------------- Phase 2: attention per head pair --------------
            # One persistent 4-bank scores tile [p, m, h, 512]; the per-j exp
            # is split into two N=1024 ops (one per m-half) so the m0 half
            # refills (scores of j+1, subtile WAR) while the ACT engine exps
            # the m1 half: the exp stream never waits a full scores round
            # trip.  ctx matmuls (N=1024 over both m, accumulated into 2-bank
            # [65, 2, 512] tiles with the softmax-denominator ones column)
            # trail at lag 3 so the previous pair's normalize has freed the
            # accumulators.  Normalize: reciprocal straight off the psum
            # denominator row, gpsimd broadcast, one mul per head; head B
            # reaches partitions 64:127 via an SBUF-to-SBUF DMA hop on the
            # idle sync queue.
            norm_tmp = {}
            with tc.tile_pool(name="pss", bufs=1, space="PSUM") as pssp, \
                 tc.tile_pool(name="psc", bufs=1, space="PSUM") as pscp:
                # two independent scores tiles (one per m-half) so the WAR
                # tracking lets scores for j+1's m0 half land while the ACT
                # engine still exps j's m1 half: the exp stream never stalls
                # a full scores round trip
                pss_m = [pssp.tile([P, 2, 512], F32, name=f"pss{m}")
                         for m in range(2)]
                scale = float(D) ** -0.5

                def normalize2(pr, pscs):
                    rbs = []
                    for head in range(2):
                        den = denp.tile([1, 2, 512], F32, tag=f"den{head}",
                                        name=f"den{head}")
                        nc.vector.tensor_copy(den[:], pscs[head][D:D + 1, :, :])
                        dr = denp.tile([1, 2, 512], F32, tag=f"dr{head}",
                                       name=f"dr{head}")
                        nc.vector.reciprocal_approx_fast(dr[:], den[:])
                        rb = denp.tile([D, 2, 512], F32, tag=f"rb{head}",
                                       name=f"rb{head}")
                        nc.gpsimd.partition_broadcast(
                            rb[:].rearrange("p a b -> p (a b)"),
                            dr[:].rearrange("p a b -> p (a b)"))
                        rbs.append(rb)
                    nc.vector.tensor_mul(
                        ctxT2[0:D, pr, :].rearrange("p (a b) -> p a b", b=512),
                        pscs[0][0:D, :, :], rbs[0][:])
                    tmpB = denp.tile([D, 2, 512], BF16, tag="tmpB")
                    nc.vector.tensor_mul(tmpB[:], pscs[1][0:D, :, :], rbs[1][:])
                    nc.sync.dma_start(
                        ctxT2[D:2 * D, pr, :],
                        tmpB[:].rearrange("p a b -> p (a b)"))
                    norm_tmp["tmpB"] = tmpB

                # ctx groups trail the exp stream by 3 slots and CARRY ACROSS
                # pair boundaries: pair p's j5..j7 ctx matmuls run inside
                # pair p+1's first slots, so the PE never idles long enough
                # at a boundary for the HAM clock gate to re-throttle
                all_pscs = {}
                exps_of = {}
                pending = []

                def ctx_mm(cpr, jc):
                    hA, hB = 2 * cpr, 2 * cpr + 1
                    for head, h in ((0, hA), (1, hB)):
                        for m in range(2):
                            nc.tensor.matmul(
                                all_pscs[cpr][head][:, m, :],
                                v_sb[:, jc, h, :],
                                exps_of[cpr][jc][:, head, m, :],
                                start=(jc == 0), stop=(jc == NT - 1))
                    if jc == NT - 1:
                        normalize2(cpr, all_pscs[cpr])

                for pr in range(NPR):
                    all_pscs[pr] = [
                        pscp.tile([D + 1, 2, 512], F32, tag=f"pc{h}",
                                  name=f"pc{h}") for h in range(2)]
                    exps_of[pr] = []
                    for j in range(NT):
                        for m in range(2):
                            for half in range(2):
                                nc.tensor.matmul(
                                    pss_m[m][:, half, :],
                                    kT_pack[half * D:(half + 1) * D, pr, bass.ts(j, P)],
                                    qT_pack[half * D:(half + 1) * D, pr, bass.ts(m, 512)],
                                    start=True, stop=True)
                        # exp tile layout [h, m, tok] so the ctx matmul's
                        # moving operand is one contiguous 1024 run; the
                        # strided side lands on the ACTIVATE output instead
                        expAB = expp.tile([P, 2, 2, 512], BF16, tag=f"expAB{j}",
                                          name=f"expAB{j}")
                        nc.scalar.activation(expAB[:, :, 0, :], pss_m[0][:],
                                             mybir.ActivationFunctionType.Exp,
                                             scale=scale)
                        nc.scalar.activation(expAB[:, :, 1, :], pss_m[1][:],
                                             mybir.ActivationFunctionType.Exp,
                                             scale=scale)
                        exps_of[pr].append(expAB)
                        pending.append((pr, j))
                        if len(pending) > 3:
                            ctx_mm(*pending.pop(0))
                while pending:
                    ctx_mm(*pending.pop(0))

                # keep the PE warm across the final normalize chain so the
                # o_proj burst doesn't start at the cold 1.2 GHz clock:
                # 3 dummies fire when the exp stream drains, 3 more hang off
                # the last normalize's tmpB (mid-chain) to bridge the gap
                for _ in range(3):
                    nc.tensor.matmul(pss_m[0][:, 0, :],
                                     xt_all[:, 0, 0, :], wq_t[0][:],
                                     start=True, stop=True)
                tmpB_last = norm_tmp["tmpB"]
                for _ in range(3):
                    nc.tensor.matmul(
                        pss_m[0][0:P, 1, :],
                        tmpB_last[:, 0, 0:P], tmpB_last[:, 0, :],
                        start=True, stop=True)

            # ---------------- Phase 3: output projection --------------------
            with tc.tile_pool(name="pso", bufs=2, space="PSUM") as psop:
                for ti in range(NT):
                    pso = psop.tile([P, C], F32, name="pso")
                    for pr in range(NPR):
                        for m in range(2):
                            nc.tensor.matmul(
                                pso[:, bass.ts(m, 512)],
                                ctxT2[:, pr, bass.ts(ti, P)],
                                wo_l[pr][:, bass.ts(m, 512)],
                                start=(pr == 0), stop=(pr == NPR - 1))
                    out_sb = finp.tile([P, C], BF16, tag="out", name="out_sb")
                    nc.vector.tensor_copy(out_sb[:], pso[:])
                    nc.sync.dma_start(out_d.ap()[bass.ts(ti, P), :], out_sb[:])

    nc.compile()
    return nc


def _rope_tables(w, b):
    """A[t,d], B[t,d] with the rotate-half sign folded into B."""
    inv_freq = 1.0 / THETA ** (np.arange(0, D, 2, dtype=np.float64) / D)
    freqs = np.arange(L, dtype=np.float64)[:, None] * inv_freq[None, :]
    freqs = np.concatenate([freqs, freqs], axis=1)           # [L, D]
    cos, sin = np.cos(freqs), np.sin(freqs)
    w = w.astype(np.float64)
    w_rot = np.concatenate([w[D // 2:], w[:D // 2]])
    sgn = np.concatenate([-np.ones(D // 2), np.ones(D // 2)])
    A = (cos * w[None, :]).astype(np.float32)
    Bt = (sin * w_rot[None, :] * sgn[None, :]).astype(np.float32)
    if np.any(b != 0):
        raise NotImplementedError("nonzero qk-norm bias not supported")
    return A, Bt


def _make_in_maps(inputs):
    from ml_dtypes import bfloat16

    x = np.asarray(inputs["q"], dtype=np.float32)
    Wq = np.asarray(inputs["Wq"], dtype=np.float32)
    Wk = np.asarray(inputs["Wk"], dtype=np.float32)
    Wv = np.asarray(inputs["Wv"], dtype=np.float32)
    Wo = np.asarray(inputs["Wo"], dtype=np.float32)
    bo = np.asarray(inputs["bo"], dtype=np.float32)
    assert not np.any(bo != 0), "nonzero output bias not supported"

    Aq, Bq = _rope_tables(np.asarray(inputs["qn_w"], np.float32),
                          np.asarray(inputs["qn_b"], np.float32))
    Ak, Bk = _rope_tables(np.asarray(inputs["kn_w"], np.float32),
                          np.asarray(inputs["kn_b"], np.float32))
    WoT = np.ascontiguousarray(Wo.T)                          # [C(c'), C(o)]

    def _tbl(a):   # [L, D] -> [P, NT, D] (partition-major, contiguous DMA)
        return np.ascontiguousarray(
            a.reshape(NT, P, D).transpose(1, 0, 2)).astype(bfloat16)
    Aqr, Bqr, Akr, Bkr = _tbl(Aq), _tbl(Bq), _tbl(Ak), _tbl(Bk)

    in_maps = []
    for c in range(8):
        b_, g = c // 2, c % 2
        sl = slice(g * CG, (g + 1) * CG)
        in_maps.append({
            "xT": np.ascontiguousarray(
                x[b_].T.reshape(NCK, P, NT, P).transpose(1, 2, 0, 3)).astype(bfloat16),
            "wqT": np.ascontiguousarray(Wq[sl, :].T).astype(bfloat16),
            "wkT": np.ascontiguousarray(Wk[sl, :].T).astype(bfloat16),
            "wvT": np.ascontiguousarray(Wv[sl, :].T).astype(bfloat16),
            # [pair, 2*D rows (= the pair's context channels), C]
            "woT": np.ascontiguousarray(
                WoT[sl, :].reshape(NPR, P, C)).astype(bfloat16),
            "aq": Aqr, "bq": Bqr, "ak": Akr, "bk": Bkr,
        })
    return in_maps


def kernel(**inputs):
    in_maps = _make_in_maps(inputs)

    if "nc" not in _NC_CACHE:
        _NC_CACHE["nc"] = _build_nc()
    nc = _NC_CACHE["nc"]

    B = 4
    res = run_bass_kernel_spmd(nc, in_maps, core_ids=list(range(8)))
    # each core wrote its full [L, C] o_proj partial; unshard = sum the two
    # head-group partials per batch
    out = np.empty((B, L, C), dtype=np.float32)
    for b_ in range(B):
        out[b_] = (res.results[2 * b_]["out"].astype(np.float32)
                   + res.results[2 * b_ + 1]["out"].astype(np.float32))
    return out

